# revision 9
# baseline (speedup 1.0000x reference)
"""MoE dispatcher kernel for Trainium2 (8 NeuronCores, expert-parallel).

Contract: kernel(**inputs) takes FULL inputs and returns the FULL output.

Strategy (expert-parallel, matches the sharding hint):
  - host: softmax(gate_logits) -> top-2 -> combine weights per (token, expert)
  - host "all-to-all dispatch": for expert e, gather its routed tokens,
    pre-scale rows by the combine weight (w * (x @ W) == (w*x) @ W), pad to a
    common capacity C, transpose to [D, C] so the device streams tokens along
    the free dim.  One expert per core.
  - device (per core): Y^T[D,C] = W[e]^T @ X^T via PE array, tiled
    [128 x <=512] PSUM accumulation over K=D.
  - host "all-to-all combine": scatter-add each expert's Y rows back to the
    token axis (plain add; weights were folded into x).

Perf structure (from NTFF trace analysis):
  - ~7.5us framework preamble before any user instruction; HWDGE DMA queues
    deliver their first bytes ~1.7us after the issue instruction
  - an engine that *reaches* a semaphore wait before the sem fires pays
    ~1.2us wake-up latency; waits that are already satisfied are free.  So:
    8 warmup matmuls keep the PE busy (HAM clock-gate ramps to 2.4GHz once)
    until the first W/X chunk sems have fired
  - W and first-n-tile X arrive in chunks sized so consumption at the warm
    matmul rate tracks arrival; a chunk is usable only when its whole DMA
    lands, so chunks are fine-grained early and coarse late
  - input DMA issues alternate W(sync-queue)/X(scalar-queue) so the 8
    round-robin completion-sem lanes are reused only by DMAs whose lane
    predecessor completed long before
  - outputs in bf16 (halves write traffic), evicted per-n-tile as ONE batched
    DMA; the last n-tile evicts per-m-tile (alternating queues) so the tail
    after the final matmul is one small DMA

DRAM layouts are host-permuted so every DMA is contiguous per partition:
  w   [P, KT*D]        w[p, k*D + d] = W[e][k*128 + p, d]
  x0  [P, KT*nsz0]     x0[p, k*nsz0 + n]   = X^T[k*128 + p, n]        (n-tile 0)
  xr  [P, KT*(C-nsz0)] j>=1 blocks packed: [j1: k-major nsz1][j2: ...]
  yt  [P, MT, C]       yt[p, m, n] = Y^T[m*128 + p, n]
"""

import os

import numpy as np

N_CORES = 8
P = 128
NSPLIT = 512  # max moving-operand / PSUM-bank free dim (fp32 out)
NMIN = 256  # keep moving tiles >=256 wide

# matmul input dtype: "float32", "float32r", or "bfloat16"
MM_DT = os.environ.get("BASS_MOE_DT", "bfloat16")
# device output dtype
OUT_DT = os.environ.get("BASS_MOE_OUT_DT", "bfloat16")
WARMUP_MM = int(os.environ.get("BASS_MOE_WARMUP", "8"))

_prog_cache: dict = {}


def _np_dt(name):
    if name == "bfloat16":
        import ml_dtypes

        return ml_dtypes.bfloat16
    return np.float32


def _n_tiles(C):
    """Split C into tiles of at most NSPLIT, all at least NMIN wide."""
    out = []
    rem = C
    n0 = 0
    while rem > 0:
        if rem > NSPLIT + NMIN // 2:
            sz = NSPLIT
        elif rem > NSPLIT:
            # split the remainder evenly (128-aligned) so both parts >= NMIN
            sz = (rem // 2 + P - 1) // P * P
        else:
            sz = rem
        out.append((n0, sz))
        n0 += sz
        rem -= sz
    return out


def _w_recs(KT, D):
    """W arrival chunks (k0, nk, d0, nd): the k=0 tile split in half by
    output column so the first matmul's data lands ASAP, per-k-tile chunks
    while the PE may still be cold, coarse 2-k-tile chunks later."""
    if KT < 8 or D % (2 * P):
        return [(k, 1, 0, D) for k in range(KT)]
    recs = [(0, 1, 0, D // 2), (0, 1, D // 2, D // 2)]
    recs += [(1, 1, 0, D), (2, 1, 0, D), (3, 1, 0, D)]
    k = 4
    while k < KT:
        nk = min(2, KT - k)
        recs.append((k, nk, 0, D))
        k += nk
    return recs


def _x0_chunks(KT):
    """First-n-tile X chunks (k0, nk) matching W arrival."""
    if KT < 8:
        return [(k, 1) for k in range(KT)]
    out = [(0, 1), (1, 3)]
    k = 4
    while k < KT:
        nk = min(4, KT - k)
        out.append((k, nk))
        k += nk
    return out


def _build_program(D: int, C: int, mm_dt_name: str, out_dt_name: str):
    import concourse.bacc as bacc
    import concourse.mybir as mybir
    import concourse.tile as tile

    mm_dt = getattr(mybir.dt, mm_dt_name)
    out_dt = getattr(mybir.dt, out_dt_name)
    KT = D // P  # k tiles (contraction)
    MT = D // P  # m tiles (output features)
    n_tiles = _n_tiles(C)
    NT = len(n_tiles)
    n0_first, nsz_first = n_tiles[0]
    CR = C - nsz_first  # columns in n-tiles 1..NT-1

    nc = bacc.Bacc(None, target_bir_lowering=False)
    x0 = nc.declare_dram_parameter("x0", [P, KT * nsz_first], mm_dt, isOutput=False)
    if CR:
        xr = nc.declare_dram_parameter("xr", [P, KT * CR], mm_dt, isOutput=False)
    w = nc.declare_dram_parameter("w", [P, KT * D], mm_dt, isOutput=False)
    yt = nc.declare_dram_parameter("yt", [P, MT, C], out_dt, isOutput=True)

    w_recs = _w_recs(KT, D)
    x0ch = _x0_chunks(KT)
    # offset of n-tile j's block inside xr (flat, per partition)
    xr_off = []
    off = 0
    for j, (n0, nsz) in enumerate(n_tiles):
        if j == 0:
            xr_off.append(0)
            continue
        xr_off.append(off)
        off += KT * nsz

    with tile.TileContext(nc) as tc:
        with (
            tc.tile_pool(name="wpool", bufs=len(w_recs)) as wpool,
            tc.tile_pool(name="xpool", bufs=len(x0ch) + 1) as xpool,
            tc.tile_pool(name="psum", bufs=8, space="PSUM") as psum_pool,
            tc.tile_pool(name="opool", bufs=2) as opool,
            tc.tile_pool(name="lastpool", bufs=4) as lastpool,
            tc.tile_pool(name="warm", bufs=1) as warmpool,
        ):
            # Input DMAs first, alternating between the two HWDGE queues in
            # consumption order so both stream from ~t=0 and completion-sem
            # lanes are reused only long after their predecessor fired.
            w_sb = [None] * len(w_recs)
            x0_sb = [None] * len(x0ch)
            xr_sb = None

            def issue_w(i):
                k0, nk, d0, nd = w_recs[i]
                tw = wpool.tile([P, nk, nd], mm_dt, tag="w_sb", name="w_sb")
                if nk == 1:
                    src = w[:, k0 * D + d0 : k0 * D + d0 + nd]
                else:
                    src = w[:, k0 * D : (k0 + nk) * D]
                nc.sync.dma_start(tw[:].rearrange("p k d -> p (k d)"), src)
                w_sb[i] = tw

            def issue_x0(i):
                k0, nk = x0ch[i]
                tx = xpool.tile([P, nk * nsz_first], mm_dt, tag="x0_sb", name="x0_sb")
                nc.scalar.dma_start(
                    tx[:], x0[:, k0 * nsz_first : (k0 + nk) * nsz_first]
                )
                x0_sb[i] = tx

            nw, nx = len(w_recs), len(x0ch)
            for i in range(max(nw, nx)):
                if i < nw:
                    issue_w(i)
                if i < nx:
                    issue_x0(i)
            if CR:
                xr_sb = xpool.tile([P, KT * CR], mm_dt, tag="xr_sb", name="xr_sb")
                nc.scalar.dma_start(xr_sb[:], xr[:, :])

            if WARMUP_MM:
                # Keep the PE busy from the first post-preamble instant: the
                # HAM clock gate ramps to 2.4GHz during the warmup and the
                # first real matmul's data sems fire before the PE arrives.
                wt = warmpool.tile([P, NSPLIT], mybir.dt.bfloat16, tag="warm_w")
                nc.vector.memset(wt[:], 0.0)
                wp = psum_pool.tile(
                    [P, NSPLIT], mybir.dt.float32, tag="ps", name="warm_ps"
                )
                for i in range(WARMUP_MM):
                    nc.tensor.matmul(
                        wp[:], lhsT=wt[:, :P], rhs=wt[:], start=True, stop=True
                    )

            def w_slice(k, mi):
                for rec, tw in zip(w_recs, w_sb):
                    k0, nk, d0, nd = rec
                    if k0 <= k < k0 + nk and d0 <= mi * P < d0 + nd:
                        return tw[:, k - k0, mi * P - d0 : mi * P - d0 + P]
                raise AssertionError

            def x_slice(j, k, nsz):
                if j == 0:
                    for (k0, nk), tx in zip(x0ch, x0_sb):
                        if k0 <= k < k0 + nk:
                            return tx[
                                :, (k - k0) * nsz_first : (k - k0) * nsz_first + nsz
                            ]
                    raise AssertionError
                o = xr_off[j] + k * nsz
                return xr_sb[:, o : o + nsz]

            def mm(ps, mi, j, k, nsz, start, stop):
                nc.tensor.matmul(
                    ps[:, :nsz],
                    lhsT=w_slice(k, mi),
                    rhs=x_slice(j, k, nsz),
                    start=start,
                    stop=stop,
                )

            # First n-tile: walk the arriving W pieces in issue order; all
            # 8 PSUM banks accumulate one m-tile each; batch-evict at the end.
            ps0 = [
                psum_pool.tile([P, NSPLIT], mybir.dt.float32, tag="ps", name="ps0")
                for _ in range(MT)
            ]
            for k0, nk, d0, nd in w_recs:
                for kl in range(nk):
                    k = k0 + kl
                    for mi in range(d0 // P, (d0 + nd) // P):
                        mm(
                            ps0[mi],
                            mi,
                            0,
                            k,
                            nsz_first,
                            start=(k == 0),
                            stop=(k == KT - 1),
                        )
            ob0 = opool.tile([P, MT, NSPLIT], out_dt, tag="ob", name="ob0")
            for mi in range(MT):
                nc.vector.tensor_copy(ob0[:, mi, :nsz_first], ps0[mi][:, :nsz_first])
            nc.sync.dma_start(
                yt[:, :, n0_first : n0_first + nsz_first],
                ob0[:, :, :nsz_first],
            )

            # Middle n-tiles: fused k loop per m-tile, one batched evict DMA.
            for j, (n0, nsz) in enumerate(n_tiles):
                if j == 0 or j == NT - 1:
                    continue
                ob = opool.tile([P, MT, NSPLIT], out_dt, tag="ob", name="ob")
                for mi in range(MT):
                    ps = psum_pool.tile(
                        [P, NSPLIT], mybir.dt.float32, tag="ps", name="ps"
                    )
                    for k in range(KT):
                        mm(ps, mi, j, k, nsz, start=(k == 0), stop=(k == KT - 1))
                    nc.vector.tensor_copy(ob[:, mi, :nsz], ps[:, :nsz])
                nc.sync.dma_start(yt[:, :, n0 : n0 + nsz], ob[:, :, :nsz])

            # Last n-tile: evict per m-tile (alternating output queues) so the
            # post-compute tail is one small DMA instead of a full n-tile.
            if NT > 1:
                n0, nsz = n_tiles[NT - 1]
                for mi in range(MT):
                    ps = psum_pool.tile(
                        [P, NSPLIT], mybir.dt.float32, tag="ps", name="ps"
                    )
                    for k in range(KT):
                        mm(ps, mi, NT - 1, k, nsz, start=(k == 0), stop=(k == KT - 1))
                    ot = lastpool.tile([P, NSPLIT], out_dt, tag="ot", name="ot")
                    nc.vector.tensor_copy(ot[:, :nsz], ps[:, :nsz])
                    eng = nc.scalar if mi % 2 == 0 else nc.sync
                    eng.dma_start(yt[:, mi, n0 : n0 + nsz], ot[:, :nsz])
    nc.compile()
    return nc


def kernel(hidden: np.ndarray, gate_logits: np.ndarray, W: np.ndarray) -> np.ndarray:
    from concourse.bass_utils import run_bass_kernel_spmd

    hidden = np.asarray(hidden)
    gate_logits = np.asarray(gate_logits)
    W = np.asarray(W)
    B, S, D = hidden.shape
    T, E = gate_logits.shape
    assert E == N_CORES
    x = np.ascontiguousarray(hidden.reshape(T, D).astype(np.float32))

    # --- routing on host (fp32, matches reference softmax/top-2) ---
    g = gate_logits.astype(np.float32)
    m = g.max(axis=-1, keepdims=True)
    p = np.exp(g - m)
    p /= p.sum(axis=-1, keepdims=True)
    top2 = np.argpartition(-p, 1, axis=-1)[:, :2]

    routed = [np.nonzero((top2 == e).any(axis=1))[0] for e in range(E)]
    counts = np.array([len(r) for r in routed])
    C = max(NMIN, int(-(-counts.max() // P)) * P)  # capacity, multiple of 128

    mm_np = _np_dt(MM_DT)
    KT = D // P
    n_tiles = _n_tiles(C)
    n0_first, nsz_first = n_tiles[0]
    CR = C - nsz_first

    in_maps = []
    for e in range(E):
        idx = routed[e]
        scale = p[idx, e].astype(np.float32)
        xe = x[idx] * scale[:, None]  # [cnt, D]
        xt_full = np.zeros((D, C), dtype=mm_np)
        xt_full[:, : len(idx)] = xe.T.astype(mm_np)
        xk = xt_full.reshape(KT, P, C)  # [KT, P, C]
        # n-tile 0: [P, KT*nsz0] k-major
        x0_dram = (
            xk[:, :, :nsz_first].transpose(1, 0, 2).reshape(P, KT * nsz_first)
        )
        im = {
            "x0": np.ascontiguousarray(x0_dram),
            "w": np.ascontiguousarray(
                W[e]
                .astype(mm_np)
                .reshape(KT, P, D)
                .transpose(1, 0, 2)
                .reshape(P, KT * D)
            ),
        }
        if CR:
            # n-tiles >=1 packed per partition: [j1: k-major][j2: ...]
            parts = []
            for j, (n0, nsz) in enumerate(n_tiles):
                if j == 0:
                    continue
                parts.append(
                    xk[:, :, n0 : n0 + nsz].transpose(1, 0, 2).reshape(P, KT * nsz)
                )
            im["xr"] = np.ascontiguousarray(np.concatenate(parts, axis=1))
        in_maps.append(im)

    key = (D, C, MM_DT, OUT_DT, WARMUP_MM)
    if key not in _prog_cache:
        _prog_cache[key] = _build_program(D, C, MM_DT, OUT_DT)
    nc = _prog_cache[key]

    res = run_bass_kernel_spmd(nc, in_maps, core_ids=list(range(N_CORES)))

    # --- combine on host ---
    out = np.zeros((T, D), dtype=np.float32)
    for e in range(E):
        idx = routed[e]
        # yt [P, MT, C] -> Y^T [D, C] with d = m*128 + p
        ye = np.asarray(res.results[e]["yt"], dtype=np.float32)
        ye_t = ye.transpose(1, 0, 2).reshape(D, C)
        out[idx] += ye_t[:, : len(idx)].T
    return out.reshape(B, S, D)
